# revision 7
# baseline (speedup 1.0000x reference)
# Trainium2 Bass kernel for nn_Decoder (LSTM decoder + GCN message passing).
#
# Strategy (8 NeuronCores, SPMD), v2:
#   * Nodes padded N=10000 -> 10240 so every core owns exactly 1280 nodes
#     = 10 dst tiles; the global source space is exactly 80 blocks of 128.
#   * LSTM is feature-major ([H, nodes]) and chunk-pipelined: 3 node chunks
#     (512/512/256) double-buffered over two 4-bank PSUM gate tiles so PE,
#     ACT and DVE overlap across chunks. Gates are packed [i|f|o|g] so ONE
#     batched sigmoid ACT covers i,f,o; gate biases are folded into an
#     augmented K=65 matmul from z-space (W_ih@W_fc2.T with a ones row),
#     which also removes the need for a separate x_proj bias add.
#   * All elementwise state (c, h, gates) is fp16 for DVE 2x mode.
#   * Y projection (h @ (W_gcn@W_fc3), 16 cols/step) reuses the freed gate
#     PSUM window right after activation; a single strided DVE copy moves
#     it into the per-core Y table.
#   * Y is shipped in 3 column slices (t=0:5, 5:10, 10:12) via AllGather as
#     soon as each finishes; a dummy collective at t=0 absorbs the ~25us
#     rendezvous barrier. Shards are shipped partition-major so assembly
#     into the SBUF Y table is 8 contiguous segments per partition.
#   * GCN scatter = block-dense matmul agg[dst] = sum_sb A[sb,dst].T @ Y[sb]
#     with A stored in fp8e4 (edge multiplicities are tiny ints, exact) and
#     PREFETCHED whole into SBUF (12.9MB) during the LSTM, so the GCN phase
#     is pure tensor work (800 matmuls of 192 cols) with no HBM traffic.
import os
import numpy as np
import ml_dtypes

import concourse.bass as bass
import concourse.bacc as bacc
import concourse.tile as tile
from concourse import mybir
from concourse import bass_utils

P = 128
N, T, NF, H, L, E = 10000, 12, 16, 128, 64, 160000
NCORES = 8
NCN_RAW = N // NCORES        # 1250 real nodes per core
NCN = 1280                   # padded nodes per core (10 full tiles)
NP = NCN * NCORES            # 10240 padded global nodes
NT = NCN // P                # 10 dst tiles per core
NSB = NP // P                # 80 source blocks
TNF = T * NF                 # 192
CH = [(0, 512), (512, 512), (1024, 256)]        # (col offset, width)
CHT = [(0, 4), (4, 4), (8, 2)]                  # (tile offset, ntiles)
SLICES = [(0, 5), (5, 10), (10, 12)]            # t-ranges of the 3 Y slices

F32 = mybir.dt.float32
F16 = mybir.dt.float16
F8 = mybir.dt.float8e4

_BUILD_CACHE = {}
LAST_RESULTS = None  # BassKernelResults of the most recent run (for test harness)


def _build():
    nc = bacc.Bacc("TRN2", target_bir_lowering=False, debug=False,
                   num_devices=NCORES)

    # ---------------- I/O declarations ----------------
    zT = nc.dram_tensor("zT", [L + 1, NCN], F16, kind="ExternalInput")
    wzg = nc.dram_tensor("wzg", [L + 1, 4 * H], F16, kind="ExternalInput")
    wfc2 = nc.dram_tensor("wfc2", [L, H], F16, kind="ExternalInput")
    b2 = nc.dram_tensor("b2", [P, 1], F32, kind="ExternalInput")
    whh = nc.dram_tensor("whh", [H, 4 * H], F16, kind="ExternalInput")
    wcomb = nc.dram_tensor("wcomb", [H, NF], F16, kind="ExternalInput")
    mdv = nc.dram_tensor("mdv", [P, NT], F32, kind="ExternalInput")
    dinvt = nc.dram_tensor("dinvt", [P, NT], F32, kind="ExternalInput")
    bout = nc.dram_tensor("bout", [P, TNF], F32, kind="ExternalInput")
    # A blocks: ablk[p, (sb*NT + kt)*P + d] = multiplicity of edge
    # (src = sb*128+p, dst_local = kt*128+d), fp8 (exact small ints).
    ablk = nc.dram_tensor("ablk", [P, NSB * NT * P], F8, kind="ExternalInput")
    xhat = nc.dram_tensor("xhat", [NCN, TNF], F32, kind="ExternalOutput")

    with tile.TileContext(nc) as tc:
        with tc.tile_pool(name="cpool", bufs=1) as cp, \
             tc.tile_pool(name="spool", bufs=1) as sp, \
             tc.tile_pool(name="dram", bufs=1, space="DRAM") as dp:

            # ---- tiny dummy collective: absorb the rendezvous barrier ----
            db_i = dp.tile([P, 1], F32, name="db_i")
            db_o = dp.tile([NCORES * P, 1], F32, addr_space="Shared",
                           name="db_o")
            dbs = cp.tile([P, 1], F32)
            nc.vector.memset(dbs[:], 0.0)
            nc.sync.dma_start(db_i[:], dbs[:])
            nc.gpsimd.collective_compute(
                "AllGather", mybir.AluOpType.bypass,
                replica_groups=[list(range(NCORES))],
                ins=[db_i.opt()], outs=[db_o.opt()])

            # ---- constant loads ----
            zt_sb = cp.tile([L + 1, NCN], F16)
            nc.sync.dma_start(zt_sb[:], zT[:])
            wzg_sb = cp.tile([L + 1, 4 * H], F16)
            nc.sync.dma_start(wzg_sb[:], wzg[:])
            wfc2_sb = cp.tile([L, H], F16)
            nc.sync.dma_start(wfc2_sb[:], wfc2[:])
            b2_sb = cp.tile([P, 1], F32)
            nc.sync.dma_start(b2_sb[:], b2[:])
            whh_sb = cp.tile([H, 4 * H], F16)
            nc.sync.dma_start(whh_sb[:], whh[:])
            wcomb_sb = cp.tile([H, NF], F16)
            nc.sync.dma_start(wcomb_sb[:], wcomb[:])
            mdv_sb = cp.tile([P, NT], F32)
            nc.sync.dma_start(mdv_sb[:], mdv[:])
            dinv_sb = cp.tile([P, NT], F32)
            nc.sync.dma_start(dinv_sb[:], dinvt[:])
            bout_sb = cp.tile([P, TNF], F32)
            nc.sync.dma_start(bout_sb[:], bout[:])

            # ---- A prefetch: whole adjacency into SBUF on gpsimd queue ----
            asb = sp.tile([P, NSB * NT * P], F8, name="asb")
            pfgate = cp.tile([P, 1], F16)

            # ---- state tensors ----
            hdT = sp.tile([H, NCN], F16)       # fc2 output == h0
            hT = sp.tile([H, NCN], F16)        # h_t (t >= 0 after write)
            cT = sp.tile([H, NCN], F16)        # c_t
            nc.vector.memset(cT[:], 0.0)
            ysb = sp.tile([P, NT * TNF], F16)  # per-core Y rows (tile-major)
            ysb_r = ysb[:].rearrange("p (k g) -> p k g", g=TNF)
            ytab = sp.tile([P, NSB * TNF], F16, name="ytab")
            ytab_r = ytab[:].rearrange("p (sb g) -> p sb g", g=TNF)

            # shipping DRAM tensors (partition-major shards)
            ysh = [dp.tile([P, NT * (t1 - t0) * NF], F16, name=f"ysh{i}")
                   for i, (t0, t1) in enumerate(SLICES)]
            yfull = [dp.tile([NCORES * P, NT * (t1 - t0) * NF], F16,
                             addr_space="Shared", name=f"yfull{i}")
                     for i, (t0, t1) in enumerate(SLICES)]

            with tc.tile_pool(name="psL", bufs=1, space="PSUM") as psL, \
                 tc.tile_pool(name="wpL", bufs=1) as wpL:
                pqA = psL.tile([P, 2048], F32, name="pqA")
                pqB = psL.tile([P, 2048], F32, name="pqB")
                pqs = [pqA, pqB]

                # ---- hd = z @ W_fc2 + b_fc2 (feature-major) ----
                for ci, (off, cw) in enumerate(CH):
                    pq = pqs[ci % 2]
                    nc.tensor.matmul(out=pq[:, :cw], lhsT=wfc2_sb[:],
                                     rhs=zt_sb[:L, off:off + cw],
                                     start=True, stop=True)
                    nc.scalar.activation(
                        out=hdT[:, off:off + cw], in_=pq[:, :cw],
                        func=mybir.ActivationFunctionType.Identity,
                        bias=b2_sb[:, :1])

                # A prefetch, gated behind hdT so the tiny critical input
                # loads get the DMA engines to themselves at startup.
                nc.gpsimd.tensor_copy(out=pfgate[:], in_=hdT[:, 0:1])
                APF = 4
                acols = NSB * NT * P
                for i in range(APF):
                    c0 = acols * i // APF
                    c1 = acols * (i + 1) // APF
                    nc.gpsimd.dma_start(asb[:, c0:c1], ablk[:, c0:c1])

                def ship_slice(si):
                    t0, t1 = SLICES[si]
                    w = (t1 - t0) * NF
                    # scale by mask*dinv[src] per dst tile, in place
                    for k in range(NT):
                        nc.vector.tensor_scalar(
                            out=ysb_r[:, k, t0 * NF:t1 * NF],
                            in0=ysb_r[:, k, t0 * NF:t1 * NF],
                            scalar1=mdv_sb[:, k:k + 1], scalar2=None,
                            op0=mybir.AluOpType.mult)
                    # ship shard (partition-major: [128, NT*w])
                    nc.sync.dma_start(
                        ysh[si][:].rearrange("p (k f) -> p k f", f=w),
                        ysb_r[:, :, t0 * NF:t1 * NF])
                    nc.gpsimd.collective_compute(
                        "AllGather", mybir.AluOpType.bypass,
                        replica_groups=[list(range(NCORES))],
                        ins=[ysh[si].opt()], outs=[yfull[si].opt()])
                    # assemble: ytab[p, (c*NT+k)*192 + t0*16+f] =
                    #   yfull[c*128+p, k*w+f]
                    for c in range(NCORES):
                        nc.sync.dma_start(
                            ytab_r[:, c * NT:(c + 1) * NT, t0 * NF:t1 * NF],
                            yfull[si][c * P:(c + 1) * P, :].rearrange(
                                "p (k f) -> p k f", f=w))

                # ---- LSTM: T steps, 3 chunk-slots per step ----
                sg_tiles = [wpL.tile([P, 2048], F16, name=f"sg{i}")
                            for i in range(2)]
                tmp_tiles = [wpL.tile([P, 512], F16, name=f"tmp{i}")
                             for i in range(2)]
                thc_tiles = [wpL.tile([P, 512], F16, name=f"thc{i}")
                             for i in range(2)]

                for t in range(T):
                    prev = hdT if t == 0 else hT
                    for ci, (off, cw) in enumerate(CH):
                        cs = t * 3 + ci
                        pq = pqs[cs % 2]
                        sg = sg_tiles[cs % 2]
                        tmp = tmp_tiles[cs % 2]
                        thc = thc_tiles[cs % 2]
                        k0, ntl = CHT[ci]
                        # gates: [i|f|o|g] windows at fixed 512-col (bank)
                        # strides -- each gate owns a 2KB PSUM bank so the
                        # start=True zero-region of one gate cannot clobber
                        # another gate's partials.
                        for q in range(4):
                            nc.tensor.matmul(
                                out=pq[:, q * 512:q * 512 + cw],
                                lhsT=wzg_sb[:, q * H:(q + 1) * H],
                                rhs=zt_sb[:, off:off + cw],
                                start=True, stop=False)
                        for q in range(4):
                            nc.tensor.matmul(
                                out=pq[:, q * 512:q * 512 + cw],
                                lhsT=whh_sb[:, q * H:(q + 1) * H],
                                rhs=prev[:, off:off + cw],
                                start=False, stop=True)
                        # activations: one sigmoid over [i,f,o], tanh for g
                        pq_q = pq[:].rearrange("p (q w) -> p q w", w=512)
                        sg_q = sg[:].rearrange("p (q w) -> p q w", w=512)
                        nc.scalar.activation(
                            out=sg_q[:, 0:3, 0:cw], in_=pq_q[:, 0:3, 0:cw],
                            func=mybir.ActivationFunctionType.Sigmoid)
                        nc.scalar.activation(
                            out=sg[:, 1536:1536 + cw],
                            in_=pq[:, 1536:1536 + cw],
                            func=mybir.ActivationFunctionType.Tanh)
                        # c = f*c + i*g ; h = o * tanh(c)   (all fp16)
                        csl = cT[:, off:off + cw]
                        nc.vector.tensor_mul(out=csl, in0=csl,
                                             in1=sg[:, 512:512 + cw])
                        nc.vector.tensor_mul(out=tmp[:, :cw],
                                             in0=sg[:, 0:cw],
                                             in1=sg[:, 1536:1536 + cw])
                        nc.vector.tensor_add(out=csl, in0=csl,
                                             in1=tmp[:, :cw])
                        nc.scalar.activation(
                            out=thc[:, :cw], in_=csl,
                            func=mybir.ActivationFunctionType.Tanh)
                        nc.vector.tensor_mul(out=hT[:, off:off + cw],
                                             in0=sg[:, 1024:1024 + cw],
                                             in1=thc[:, :cw])
                        # Y projection for this chunk-step into freed PSUM.
                        # One accumulation group (start only on j=0): all
                        # windows share bank 0 and a second start=True would
                        # re-zero the bank, wiping earlier tiles' outputs.
                        for j in range(ntl):
                            nc.tensor.matmul(
                                out=pq[:, j * NF:(j + 1) * NF],
                                lhsT=hT[:, off + j * P:off + (j + 1) * P],
                                rhs=wcomb_sb[:],
                                start=(j == 0), stop=(j == ntl - 1))
                        nc.vector.tensor_copy(
                            out=ysb_r[:, k0:k0 + ntl, t * NF:(t + 1) * NF],
                            in_=pq[:, :ntl * NF].rearrange(
                                "p (k f) -> p k f", f=NF))
                    for si, (t0, t1) in enumerate(SLICES):
                        if t == t1 - 1:
                            ship_slice(si)

            # ---- GCN: agg[kt] = sum_sb A[sb,kt].T @ Y[sb] ----
            with tc.tile_pool(name="psC", bufs=1, space="PSUM") as psC, \
                 tc.tile_pool(name="wpC", bufs=1) as wpC:
                for kt in range(NT):
                    pa = psC.tile([P, TNF], F32, tag="pa", bufs=2)
                    for sb in range(NSB):
                        base = (sb * NT + kt) * P
                        nc.tensor.matmul(
                            out=pa[:],
                            lhsT=asb[:, base:base + P],
                            rhs=ytab[:, sb * TNF:(sb + 1) * TNF],
                            start=(sb == 0), stop=(sb == NSB - 1))
                    osb = wpC.tile([P, TNF], F32, tag="osb", bufs=2)
                    nc.vector.scalar_tensor_tensor(
                        out=osb[:], in0=pa[:],
                        scalar=dinv_sb[:, kt:kt + 1], in1=bout_sb[:],
                        op0=mybir.AluOpType.mult,
                        op1=mybir.AluOpType.add)
                    nc.sync.dma_start(xhat[kt * P:(kt + 1) * P, :], osb[:])

    nc.compile()
    return nc


def _preprocess(z, edge_index, x_mask, W_fc2, b_fc2, W_ih, W_hh, b_ih, b_hh,
                W_gcn, b_gcn, W_fc3, b_fc3):
    z = np.asarray(z, np.float32)
    edge_index = np.asarray(edge_index).astype(np.int64)
    x_mask = np.asarray(x_mask)
    W_fc2 = np.asarray(W_fc2, np.float32)
    b_fc2 = np.asarray(b_fc2, np.float32)
    W_ih = np.asarray(W_ih, np.float32)
    W_hh = np.asarray(W_hh, np.float32)
    b_ih = np.asarray(b_ih, np.float32)
    b_hh = np.asarray(b_hh, np.float32)
    W_gcn = np.asarray(W_gcn, np.float32)
    b_gcn = np.asarray(b_gcn, np.float32)
    W_fc3 = np.asarray(W_fc3, np.float32)
    b_fc3 = np.asarray(b_fc3, np.float32)

    src = edge_index[0]
    dst = edge_index[1]
    deg = np.bincount(dst, minlength=N) + 1.0
    dinv = (1.0 / np.sqrt(deg)).astype(np.float32)
    node_mask = x_mask.any(axis=(1, 2)).astype(np.float32)

    # padded global ids: core = n // 1250, padded = core*1280 + n % 1250
    def pad_id(n):
        return (n // NCN_RAW) * NCN + (n % NCN_RAW)

    src_all = np.concatenate([src, np.arange(N, dtype=np.int64)])
    dst_all = np.concatenate([dst, np.arange(N, dtype=np.int64)])
    psrc = pad_id(src_all)
    pdst = pad_id(dst_all)

    # gate order [i, f, o, g] (pytorch order is i, f, g, o)
    perm = np.concatenate([np.arange(0, 128), np.arange(128, 256),
                           np.arange(384, 512), np.arange(256, 384)])
    Wz = (W_ih @ W_fc2.T)[perm]                       # [4H, L]
    btot = (W_ih @ b_fc2 + b_ih + b_hh)[perm]         # [4H]
    wzg_t = np.concatenate([Wz.T, btot[None, :]], axis=0).astype(np.float16)
    whh_t = np.ascontiguousarray(W_hh[perm].T.astype(np.float16))
    wfc2_t = np.ascontiguousarray(W_fc2.astype(np.float16))
    b2_t = np.ascontiguousarray(b_fc2.reshape(P, 1))
    Wcomb = np.ascontiguousarray((W_gcn @ W_fc3).astype(np.float16))
    bias = b_gcn @ W_fc3 + b_fc3
    bout_t = np.ascontiguousarray(
        np.tile(bias, (P, T)).astype(np.float32))

    in_maps = []
    acols = NSB * NT * P
    for c in range(NCORES):
        sl = slice(c * NCN_RAW, (c + 1) * NCN_RAW)
        zt_c = np.zeros((L + 1, NCN), np.float16)
        zt_c[:L, :NCN_RAW] = z[sl].T
        zt_c[L, :] = 1.0

        dv_c = np.zeros(NCN, np.float32)
        dv_c[:NCN_RAW] = dinv[sl]
        mk_c = np.zeros(NCN, np.float32)
        mk_c[:NCN_RAW] = node_mask[sl]
        mdv_t = np.ascontiguousarray((dv_c * mk_c).reshape(NT, P).T)
        dinv_t = np.ascontiguousarray(dv_c.reshape(NT, P).T)

        m = (pdst // NCN) == c
        s = psrc[m]
        ld = pdst[m] % NCN
        lin = (s % P) * (NSB * NT * P) + ((s // P) * NT + ld // P) * P \
            + (ld % P)
        counts = np.bincount(lin, minlength=P * acols)
        ablk_c = counts.reshape(P, acols).astype(ml_dtypes.float8_e4m3fn)

        in_maps.append({
            "zT": zt_c,
            "wzg": wzg_t,
            "wfc2": wfc2_t,
            "b2": b2_t,
            "whh": whh_t,
            "wcomb": Wcomb,
            "mdv": mdv_t,
            "dinvt": dinv_t,
            "bout": bout_t,
            "ablk": ablk_c,
        })
    return in_maps


def kernel(z, edge_index, x_mask, W_fc2, b_fc2, W_ih, W_hh, b_ih, b_hh,
           W_gcn, b_gcn, W_fc3, b_fc3):
    global LAST_RESULTS
    in_maps = _preprocess(z, edge_index, x_mask, W_fc2, b_fc2,
                          W_ih, W_hh, b_ih, b_hh,
                          W_gcn, b_gcn, W_fc3, b_fc3)
    if "nc" not in _BUILD_CACHE:
        _BUILD_CACHE["nc"] = _build()
    nc = _BUILD_CACHE["nc"]

    trace = bool(int(os.environ.get("KERNEL_TRACE", "0")))
    res = bass_utils.run_bass_kernel_spmd(
        nc, in_maps, core_ids=list(range(NCORES)), trace=trace)
    LAST_RESULTS = res

    out = np.empty((N, T, NF), np.float32)
    for c in range(NCORES):
        out[c * NCN_RAW:(c + 1) * NCN_RAW] = \
            res.results[c]["xhat"][:NCN_RAW].reshape(NCN_RAW, T, NF)
    return out
